# revision 1
# baseline (speedup 1.0000x reference)
"""Trainium2 Bass kernel for nn_CLRBP_23124103922240.

Math: scores[b, cls] = x[b] . W[cls] + bias[cls], softmax over 16 classes,
where W[cls] = g * tile4x4(u1 u1^T - v1 v1^T) + (1-g) * (u2 u2^T - v2 v2^T).

Key identities used:
  - tile4x4(A)[m, n] = A[m % 64, n % 64], so <X, tile(u u^T)> = uh^T X uh with
    uh = tile(u, 4); equivalently u^T (P^T X P) u with P[m, i] = (m % 64 == i).
  - v^T X v is invariant under X -> X^T, so contraction can run down X's rows.

Per sample (X = inputs[b], [256, 256], split into two 128-row chunks mc):
  stage 1 (PE): YY = [P | V2]^T X  -> YY[0:64]  = Xr  (row-pooled X, [64, 256])
                                      YY[64:96] = Y2 = V2^T X   ([32, 256])
  stage 1b (PE): Y1 = V1^T Xrp where Xrp = col-pooled Xr, done by accumulating
                 4 matmuls over 64-column slices of Xr (batched 4 samples).
  stage 2 (DVE): R1[k, b] = sum_j Y1[k, j] * (s_k V1[j, k])   (fused ttr)
                 R2[k, b] = sum_n Y2[k, n] * (s_k V2[n, k])
  stage 3 (PE): scores = R1^T G1 + R2^T G2 + 1^T b  -> [128 samples, 16]
  softmax (DVE/ACT) and DMA out.

Data-parallel over 8 NeuronCores: batch 1024 -> 128 per core.
"""

import os
import numpy as np

import concourse.bacc as bacc
import concourse.mybir as mybir
import concourse.tile as tile
from concourse.bass_utils import run_bass_kernel_spmd

N_CORES = 8
B, D, VIEW, C = 1024, 256, 4, 16
BL = B // N_CORES  # 128 samples per core
SG = 4             # samples per DMA group
NG = BL // SG      # 32 groups
F32 = mybir.dt.float32
F32R = mybir.dt.float32r

AOP = mybir.AluOpType
AFT = mybir.ActivationFunctionType
AXL = mybir.AxisListType

_cache = {}


def _build(mm_fast: bool, reps: int = 1, sg: int = SG, xbufs: int = 6,
           yybufs: int = 4):
    """Build + compile the SPMD program. mm_fast=True uses float32r matmuls
    (1 cyc/row at N>=256 vs 4 for fp32)."""
    key = (mm_fast, reps, sg, xbufs, yybufs)
    if key in _cache:
        return _cache[key]
    ng = BL // sg

    DTX = F32R if mm_fast else F32   # dtype for matmul operands

    nc = bacc.Bacc("TRN2", target_bir_lowering=False, debug=False,
                   num_devices=N_CORES)

    x_d = nc.dram_tensor("x", [128, BL, 2, 256], DTX, kind="ExternalInput").ap()
    ca_d = nc.dram_tensor("ca", [128, 384], DTX, kind="ExternalInput").ap()
    cb_d = nc.dram_tensor("cb", [128, 496], F32, kind="ExternalInput").ap()
    out_d = nc.dram_tensor("probs", [BL, C], F32, kind="ExternalOutput").ap()

    with tile.TileContext(nc) as tc:
        with (
            tc.tile_pool(name="consts", bufs=1) as consts,
            tc.tile_pool(name="xp", bufs=xbufs) as xpool,
            tc.tile_pool(name="xr", bufs=3) as xrpool,
            tc.tile_pool(name="scr", bufs=3) as scrpool,
            tc.tile_pool(name="fin", bufs=1) as fin,
            tc.tile_pool(name="yy", bufs=yybufs, space="PSUM") as yypool,
            tc.tile_pool(name="y1", bufs=2, space="PSUM") as y1pool,
            tc.tile_pool(name="sc", bufs=1, space="PSUM") as scpool,
        ):
            # group schedule: mostly sg-sample groups, 2-sample tail
            # groups to shorten the end-of-kernel drain
            sizes = [sg] * ((BL - 4) // sg) + [2, 2]
            starts = [sum(sizes[:i]) for i in range(len(sizes))]

            # issue the first two X loads before anything else so the DMA
            # stream starts at t=0; the packed const loads overlap on other
            # queues
            pre = {}
            for grp in range(2):
                xt = xpool.tile([128, sizes[grp], 2, 256], DTX, tag="xt")
                nc.sync.dma_start(
                    out=xt, in_=x_d[:, starts[grp]:starts[grp] + sizes[grp]])
                pre[grp] = xt

            # ---- constants (2 packed DMAs) ----
            # ca: [vp (2x128) | v1s (128)]; cb: [v1pt | v2pt | g1 | g2 | bo]
            ca = consts.tile([128, 384], DTX)
            nc.sync.dma_start(out=ca, in_=ca_d)
            cb = consts.tile([128, 496], F32)
            nc.sync.dma_start(out=cb, in_=cb_d)
            vp = ca[:, 0:256].rearrange("p (m c) -> p m c", m=2)
            v1s = ca[:, 256:384]
            v1pt = cb[:, 0:64]
            v2pt = cb[0:32, 64:320]
            g1 = cb[:, 320:336]
            g2 = cb[0:32, 336:352]
            bo = cb[0:1, 352:496]

            r1 = consts.tile([128, BL], F32)   # per-rank-1-term partial scores
            r2 = consts.tile([32, BL], F32)

            for rep in range(reps):
              for grp in range(len(sizes)):
                s0 = starts[grp]
                sgi = sizes[grp]
                if rep == 0 and grp in pre:
                    xt = pre[grp]
                else:
                    xt = xpool.tile([128, sgi, 2, 256], DTX, tag="xt")
                    nc.sync.dma_start(out=xt, in_=x_d[:, s0:s0 + sgi, :, :])

                xr4 = xrpool.tile([128, sgi, 256], DTX, tag="xr4")
                for si in range(sgi):
                    s = s0 + si
                    yy = yypool.tile([128, 256], F32)
                    nc.tensor.matmul(yy, vp[:, 0, :], xt[:, si, 0, :],
                                     start=True, stop=False)
                    nc.tensor.matmul(yy, vp[:, 1, :], xt[:, si, 1, :],
                                     start=False, stop=True)
                    # Xr (row-pooled X) -> SBUF for the stage-1b matmul
                    nc.scalar.copy(xr4[64:128, si, :], yy[64:128, :])
                    # path 2 reduce: R2[k, s] = sum_n Y2[k, n] * V2pT[k, n]
                    scr2 = scrpool.tile([32, 256], F32, tag="scr2")
                    nc.vector.scalar_tensor_tensor(
                        out=scr2, in0=yy[0:32, :], scalar=1.0, in1=v2pt,
                        op0=AOP.mult, op1=AOP.mult,
                        accum_out=r2[:, s:s + 1])

                # stage 1b: Y1 = V1^T Xrp, col-pool via 4 accumulated matmuls,
                # 4 samples batched in the moving operand (N=256)
                y14 = y1pool.tile([128, sgi, 64], F32,
                                  tag="y14")
                for q in range(4):
                    nc.tensor.matmul(y14, v1s[64:128, :],
                                     xr4[64:128, :, q * 64:(q + 1) * 64],
                                     start=(q == 0), stop=(q == 3))
                for si in range(sgi):
                    s = s0 + si
                    scr1 = scrpool.tile([128, 64], F32, tag="scr1")
                    nc.vector.scalar_tensor_tensor(
                        out=scr1, in0=y14[:, si, :], scalar=1.0, in1=v1pt,
                        op0=AOP.mult, op1=AOP.mult,
                        accum_out=r1[:, s:s + 1])

            # ---- stage 3: scores [128 samples, 16] ----
            sc = scpool.tile([BL, C], F32)
            nc.tensor.matmul(sc, r1, g1, start=True, stop=False)
            nc.tensor.matmul(sc, r2, g2, start=False, stop=False)
            nc.tensor.matmul(sc, bo[:, 0:128], bo[:, 128:144],
                             start=False, stop=True)

            # ---- softmax over the 16 free elements ----
            negmax = fin.tile([BL, 1], F32)
            nc.vector.tensor_reduce(out=negmax, in_=sc, axis=AXL.X,
                                    op=AOP.max, negate=True)
            e = fin.tile([BL, C], F32)
            sume = fin.tile([BL, 1], F32)
            nc.scalar.activation(out=e, in_=sc, func=AFT.Exp, bias=negmax,
                                 scale=1.0, accum_out=sume)
            rec = fin.tile([BL, 1], F32)
            nc.vector.reciprocal(rec, sume)
            probs = fin.tile([BL, C], F32)
            nc.vector.tensor_scalar_mul(probs, e, rec)
            nc.sync.dma_start(out=out_d, in_=probs)

    nc.compile()
    _cache[key] = nc
    return nc


def _host_prep(inputs, w1, w2, l, b):
    inputs = np.asarray(inputs, dtype=np.float32)
    w1 = np.asarray(w1, dtype=np.float32)
    w2 = np.asarray(w2, dtype=np.float32)
    l = np.asarray(l, dtype=np.float32)
    b = np.asarray(b, dtype=np.float32)

    g = float(1.0 / (1.0 + np.exp(-np.float32(l[0]))))

    # path 1: rank-8 factors on the 64-block; col k = cls*8 + r
    u1, v1 = w1[:, :, 4:], w1[:, :, :4]               # [16, 64, 4]
    v1cols = np.concatenate([u1, v1], axis=2)          # [16, 64, 8]
    v1small = np.ascontiguousarray(
        v1cols.transpose(1, 0, 2).reshape(64, 128)).astype(np.float32)
    s1 = np.tile(np.array([g] * 4 + [-g] * 4, np.float32), C)        # [128]
    v1pt = np.ascontiguousarray(v1small.T * s1[:, None]).astype(np.float32)

    # path 2: rank-2 factors on full d; col k = cls*2 + {u, v}
    u2, v2 = w2[:, :, 1:2], w2[:, :, 0:1]              # [16, 256, 1]
    v2cols = np.concatenate([u2, v2], axis=2)          # [16, 256, 2]
    v2full = np.ascontiguousarray(
        v2cols.transpose(1, 0, 2).reshape(256, 32)).astype(np.float32)
    s2 = np.tile(np.array([1.0 - g, -(1.0 - g)], np.float32), C)     # [32]
    v2pt = np.ascontiguousarray(v2full.T * s2[:, None]).astype(np.float32)

    # combined stationary operand [V2 | 0 | P] per row-chunk
    P = (np.arange(128)[:, None] % 64 == np.arange(64)[None, :]).astype(
        np.float32)
    vp = np.zeros((128, 2, 128), np.float32)
    for mc in range(2):
        vp[:, mc, 0:32] = v2full[mc * 128:(mc + 1) * 128, :]
        vp[:, mc, 64:128] = P

    g1 = (np.arange(128)[:, None] // 8 ==
          np.arange(C)[None, :]).astype(np.float32)
    g2 = (np.arange(32)[:, None] // 2 ==
          np.arange(C)[None, :]).astype(np.float32)

    ca = np.zeros((128, 384), np.float32)
    ca[:, 0:256] = vp.reshape(128, 256)
    ca[64:128, 256:384] = v1small
    cb = np.zeros((128, 496), np.float32)
    cb[:, 0:64] = v1pt
    cb[0:32, 64:320] = v2pt
    cb[:, 320:336] = g1
    cb[0:32, 336:352] = g2
    cb[0, 352:480] = 1.0
    cb[0, 480:496] = b

    # shard + relayout inputs: (core, p, s, mc, n)
    xs = inputs.reshape(N_CORES, BL, 2, 128, 256).transpose(0, 3, 1, 2, 4)

    shared = dict(ca=ca, cb=cb)
    in_maps = []
    for core in range(N_CORES):
        m = dict(shared)
        m["x"] = np.ascontiguousarray(xs[core])
        in_maps.append(m)
    return in_maps


def kernel(inputs, w1, w2, l, b, _trace=False, _mm_fast=None):
    if _mm_fast is None:
        _mm_fast = os.environ.get("NN_MM_DTYPE", "f32r") != "f32"
    nc = _build(_mm_fast)
    in_maps = _host_prep(inputs, w1, w2, l, b)
    res = run_bass_kernel_spmd(nc, in_maps, core_ids=list(range(N_CORES)),
                               trace=_trace)
    out = np.concatenate([r["probs"] for r in res.results], axis=0)
    if _trace:
        kernel.last_results = res
    return out



# revision 4
# speedup vs baseline: 1.2873x; 1.2873x over previous
"""Trainium2 Bass kernel for nn_CLRBP_23124103922240.

Math: scores[b, cls] = x[b] . W[cls] + bias[cls], softmax over 16 classes,
where W[cls] = g * tile4x4(u1 u1^T - v1 v1^T) + (1-g) * (u2 u2^T - v2 v2^T).

Key identities used:
  - tile4x4(A)[m, n] = A[m % 64, n % 64], so <X, tile(u u^T)> = uh^T X uh with
    uh = tile(u, 4); equivalently u^T (P^T X P) u with P[m, i] = (m % 64 == i).
  - v^T X v is invariant under X -> X^T, so contraction can run down X's rows.

X is shipped to the device in fp16 (host cast): logits have std ~530 and the
fp16 mantissa keeps the end-to-end rel err ~5e-3, well inside the 2e-2 gate,
while halving HBM traffic (the kernel is memory-bound).

Per sample (X = inputs[b], [256, 256], split into two 128-row chunks mc):
  stage 1 (PE): YY = [V2 | P]^T X -> YY[0:32]   = Y2 = V2^T X  ([32, 256])
                                     YY[64:128] = Xr (row-pooled X, [64, 256])
  stage 1b (PE): Y1 = V1^T Xrp where Xrp = col-pooled Xr, done by accumulating
                 4 matmuls over 64-column slices of Xr (batched 4 samples).
  stage 2: R2[k, b] = sum_n Y2[k, n] * (s_k V2[n, k])  (Pool engine, fused)
           R1[k, b] = sum_j Y1[k, j] * (s_k V1[j, k])  (DVE, fused)
  stage 3 (PE): scores = R1^T G1 + R2^T G2 + 1^T b  -> [128 samples, 16]
  softmax (DVE/ACT) and DMA out.

Engine balance per core (128 samples): DMA ~47us (fp16 X), PE ~41us,
ACT ~37us (pair-batched Xr copies), Pool ~40us (path-2 reduce),
DVE ~25us (path-1 reduce).

Data-parallel over 8 NeuronCores: batch 1024 -> 128 per core.
"""

import os
import numpy as np

import concourse.bacc as bacc
import concourse.mybir as mybir
import concourse.tile as tile
from concourse.bass_utils import run_bass_kernel_spmd

N_CORES = 8
B, D, VIEW, C = 1024, 256, 4, 16
BL = B // N_CORES  # 128 samples per core
SG = 4             # samples per DMA group
F32 = mybir.dt.float32
F16 = mybir.dt.float16

AOP = mybir.AluOpType
AFT = mybir.ActivationFunctionType
AXL = mybir.AxisListType

_cache = {}


def _build(sg: int = SG, xbufs: int = 6, yybufs: int = 3):
    key = (sg, xbufs, yybufs)
    if key in _cache:
        return _cache[key]

    nc = bacc.Bacc("TRN2", target_bir_lowering=False, debug=False,
                   num_devices=N_CORES)

    x_d = nc.dram_tensor("x", [128, BL, 2, 256], F16, kind="ExternalInput").ap()
    ca_d = nc.dram_tensor("ca", [128, 384], F16, kind="ExternalInput").ap()
    cb_d = nc.dram_tensor("cb", [128, 496], F32, kind="ExternalInput").ap()
    out_d = nc.dram_tensor("probs", [BL, C], F32, kind="ExternalOutput").ap()

    with tile.TileContext(nc) as tc:
        with (
            tc.tile_pool(name="consts", bufs=1) as consts,
            tc.tile_pool(name="xp", bufs=xbufs) as xpool,
            tc.tile_pool(name="xr", bufs=3) as xrpool,
            tc.tile_pool(name="scr", bufs=3) as scrpool,
            tc.tile_pool(name="fin", bufs=1) as fin,
            tc.tile_pool(name="yy", bufs=yybufs, space="PSUM") as yypool,
            tc.tile_pool(name="y1", bufs=2, space="PSUM") as y1pool,
            tc.tile_pool(name="sc", bufs=1, space="PSUM") as scpool,
        ):
            # group schedule: mostly sg-sample groups, 2-sample tail
            # groups to shorten the end-of-kernel drain
            sizes = [sg] * ((BL - 4) // sg) + [2, 2]
            starts = [sum(sizes[:i]) for i in range(len(sizes))]

            # issue the first two X loads before anything else so the DMA
            # stream starts at t=0; the packed const loads overlap on other
            # queues
            pre = {}
            for grp in range(2):
                xt = xpool.tile([128, sizes[grp], 2, 256], F16, tag="xt")
                nc.sync.dma_start(
                    out=xt, in_=x_d[:, starts[grp]:starts[grp] + sizes[grp]])
                pre[grp] = xt

            # ---- constants (2 packed DMAs) ----
            # ca: [vp (2x128) | v1s (128)]; cb: [v1pt | v2pt | g1 | g2 | bo]
            ca = consts.tile([128, 384], F16)
            nc.sync.dma_start(out=ca, in_=ca_d)
            cb = consts.tile([128, 496], F32)
            nc.sync.dma_start(out=cb, in_=cb_d)
            vp = ca[:, 0:256].rearrange("p (m c) -> p m c", m=2)
            v1s = ca[:, 256:384]
            v1pt = cb[:, 0:64]
            v2pt = cb[0:32, 64:320]
            g1 = cb[:, 320:336]
            g2 = cb[0:32, 336:352]
            bo = cb[0:1, 352:496]

            r1 = consts.tile([128, BL], F32)   # per-rank-1-term partial scores
            r2 = consts.tile([32, BL], F32)

            for grp in range(len(sizes)):
                s0 = starts[grp]
                sgi = sizes[grp]
                if grp in pre:
                    xt = pre[grp]
                else:
                    xt = xpool.tile([128, sgi, 2, 256], F16, tag="xt")
                    nc.sync.dma_start(out=xt, in_=x_d[:, s0:s0 + sgi, :, :])

                xr4 = xrpool.tile([128, sgi, 256], F16, tag="xr4")
                for pair in range(sgi // 2):
                    yy2 = yypool.tile([128, 2, 256], F32)
                    for si2 in range(2):
                        si = pair * 2 + si2
                        nc.tensor.matmul(yy2[:, si2, :], vp[:, 0, :],
                                         xt[:, si, 0, :],
                                         start=True, stop=False)
                        nc.tensor.matmul(yy2[:, si2, :], vp[:, 1, :],
                                         xt[:, si, 1, :],
                                         start=False, stop=True)
                    # One pair-batched PSUM->SBUF copy of the whole yy2
                    # (ACT cost depends on free size only, not partitions):
                    # rows 64:128 = Xr feed the stage-1b matmul, rows 0:32
                    # = Y2 feed the Pool reduce (GPSIMD can't read PSUM).
                    nc.scalar.copy(xr4[:, pair * 2:pair * 2 + 2, :],
                                   yy2[:, :, :])
                    # path 2 reduce: R2[k, s] = sum_n Y2[k,n]*V2pT[k,n]
                    for si2 in range(2):
                        s = s0 + pair * 2 + si2
                        si = pair * 2 + si2
                        scr2 = scrpool.tile([32, 256], F16, tag="scr2")
                        nc.vector.scalar_tensor_tensor(
                            out=scr2, in0=xr4[0:32, si, :], scalar=1.0,
                            in1=v2pt, op0=AOP.mult, op1=AOP.mult,
                            accum_out=r2[:, s:s + 1])

                # stage 1b: Y1 = V1^T Xrp, col-pool via 4 accumulated matmuls,
                # sgi samples batched in the moving operand
                y14 = y1pool.tile([128, sgi, 64], F32, tag="y14")
                for q in range(4):
                    nc.tensor.matmul(y14, v1s[64:128, :],
                                     xr4[64:128, :, q * 64:(q + 1) * 64],
                                     start=(q == 0), stop=(q == 3))
                for si in range(sgi):
                    s = s0 + si
                    scr1 = scrpool.tile([128, 64], F32, tag="scr1")
                    nc.vector.scalar_tensor_tensor(
                        out=scr1, in0=y14[:, si, :], scalar=1.0, in1=v1pt,
                        op0=AOP.mult, op1=AOP.mult,
                        accum_out=r1[:, s:s + 1])

            # ---- stage 3: scores [128 samples, 16] ----
            sc = scpool.tile([BL, C], F32)
            nc.tensor.matmul(sc, r1, g1, start=True, stop=False)
            nc.tensor.matmul(sc, r2, g2, start=False, stop=False)
            nc.tensor.matmul(sc, bo[:, 0:128], bo[:, 128:144],
                             start=False, stop=True)

            # ---- softmax over the 16 free elements ----
            negmax = fin.tile([BL, 1], F32)
            nc.vector.tensor_reduce(out=negmax, in_=sc, axis=AXL.X,
                                    op=AOP.max, negate=True)
            e = fin.tile([BL, C], F32)
            sume = fin.tile([BL, 1], F32)
            nc.scalar.activation(out=e, in_=sc, func=AFT.Exp, bias=negmax,
                                 scale=1.0, accum_out=sume)
            rec = fin.tile([BL, 1], F32)
            nc.vector.reciprocal(rec, sume)
            probs = fin.tile([BL, C], F32)
            nc.vector.tensor_scalar_mul(probs, e, rec)
            nc.sync.dma_start(out=out_d, in_=probs)

    nc.compile()
    _cache[key] = nc
    return nc


def _host_prep(inputs, w1, w2, l, b):
    inputs = np.asarray(inputs, dtype=np.float32)
    w1 = np.asarray(w1, dtype=np.float32)
    w2 = np.asarray(w2, dtype=np.float32)
    l = np.asarray(l, dtype=np.float32)
    b = np.asarray(b, dtype=np.float32)

    g = float(1.0 / (1.0 + np.exp(-np.float32(l[0]))))

    # path 1: rank-8 factors on the 64-block; col k = cls*8 + r
    u1, v1 = w1[:, :, 4:], w1[:, :, :4]               # [16, 64, 4]
    v1cols = np.concatenate([u1, v1], axis=2)          # [16, 64, 8]
    v1small = np.ascontiguousarray(
        v1cols.transpose(1, 0, 2).reshape(64, 128)).astype(np.float32)
    s1 = np.tile(np.array([g] * 4 + [-g] * 4, np.float32), C)        # [128]
    v1pt = np.ascontiguousarray(v1small.T * s1[:, None]).astype(np.float32)

    # path 2: rank-2 factors on full d; col k = cls*2 + {u, v}
    u2, v2 = w2[:, :, 1:2], w2[:, :, 0:1]              # [16, 256, 1]
    v2cols = np.concatenate([u2, v2], axis=2)          # [16, 256, 2]
    v2full = np.ascontiguousarray(
        v2cols.transpose(1, 0, 2).reshape(256, 32)).astype(np.float32)
    s2 = np.tile(np.array([1.0 - g, -(1.0 - g)], np.float32), C)     # [32]
    v2pt = np.ascontiguousarray(v2full.T * s2[:, None]).astype(np.float32)

    # combined stationary operand [V2 | 0 | P] per row-chunk
    P = (np.arange(128)[:, None] % 64 == np.arange(64)[None, :]).astype(
        np.float32)
    vp = np.zeros((128, 2, 128), np.float32)
    for mc in range(2):
        vp[:, mc, 0:32] = v2full[mc * 128:(mc + 1) * 128, :]
        vp[:, mc, 64:128] = P

    g1 = (np.arange(128)[:, None] // 8 ==
          np.arange(C)[None, :]).astype(np.float32)
    g2 = (np.arange(32)[:, None] // 2 ==
          np.arange(C)[None, :]).astype(np.float32)

    ca = np.zeros((128, 384), np.float16)
    ca[:, 0:256] = vp.reshape(128, 256).astype(np.float16)
    ca[64:128, 256:384] = v1small.astype(np.float16)
    cb = np.zeros((128, 496), np.float32)
    cb[:, 0:64] = v1pt
    cb[0:32, 64:320] = v2pt
    cb[:, 320:336] = g1
    cb[0:32, 336:352] = g2
    cb[0, 352:480] = 1.0
    cb[0, 480:496] = b

    # shard + relayout inputs: (core, p, s, mc, n), cast to fp16
    xs = inputs.reshape(N_CORES, BL, 2, 128, 256).transpose(0, 3, 1, 2, 4)

    shared = dict(ca=ca, cb=cb)
    in_maps = []
    for core in range(N_CORES):
        m = dict(shared)
        m["x"] = np.ascontiguousarray(xs[core]).astype(np.float16)
        in_maps.append(m)
    return in_maps


def kernel(inputs, w1, w2, l, b, _trace=False, _mm_fast=None):
    nc = _build()
    in_maps = _host_prep(inputs, w1, w2, l, b)
    res = run_bass_kernel_spmd(nc, in_maps, core_ids=list(range(N_CORES)),
                               trace=_trace)
    out = np.concatenate([r["probs"] for r in res.results], axis=0)
    if _trace:
        kernel.last_results = res
    return out


# revision 11
# speedup vs baseline: 1.4485x; 1.1253x over previous
"""Trainium2 Bass kernel for nn_CLRBP_23124103922240.

Math: scores[b, cls] = x[b] . W[cls] + bias[cls], softmax over 16 classes,
where W[cls] = g * tile4x4(u1 u1^T - v1 v1^T) + (1-g) * (u2 u2^T - v2 v2^T).

Key identities used:
  - tile4x4(A)[m, n] = A[m % 64, n % 64], so <X, tile(u u^T)> = uh^T X uh with
    uh = tile(u, 4); equivalently u^T (P^T X P) u with P[m, i] = (m % 64 == i).
  - v^T X v is invariant under X -> X^T, so contraction can run down X's rows.

X is shipped to the device in fp16 (host cast): logits have std ~530 and the
fp16 mantissa keeps the end-to-end rel err ~5e-3, well inside the 2e-2 gate,
while halving HBM traffic (the kernel is memory-bound: 16 MB/core).

Per PAIR of samples (even e, odd o; X = inputs[b] as two 128-row chunks mc):
  stage 1 (PE): yy2[:, e] = [V2 | 0 | P]^T X_e, yy2[:, o] = [0 | V2 | P]^T X_o
    -> rows 0:32 = Y2_e, rows 32:64 = Y2_o (each slot zero in the other
    sample's column block), rows 64:128 = Xr (row-pooled X) per sample.
  copy (ACT): one pair-batched PSUM->SBUF fp16 copy of all of yy2.
  path 2 (DVE): ONE fused multiply-accumulate over [64 part, 2, 256]:
    r2z[k, 2p+1] = sum Y2_pair . v2q  (both samples at once; the zero blocks
    make the masking automatic).  r2z is [64, 129] zero-initialized; its
    shifted column views r2z[0:32, 1:129] / r2z[32:64, 0:128] ARE the
    even/odd per-sample R2 operands for stage 3 (odd columns hold data,
    even columns stay zero), so no un-interleave pass is needed.
  stage 1b (PE, per 16-sample macro): Y1 = V1^T Xrp, col-pool via 4
    accumulated matmuls over 64-column slices of Xr.
  path 1 (DVE, per macro): z1 = y14 * v1q (tensor_tensor), then
    tensor_reduce over j -> R1[k, s] for 16 samples in two ops.
  stage 3 (PE): scores = R1^T G1 + R2even^T G2 + R2odd^T G2 + 1^T b.
  softmax (DVE/ACT) and DMA out.

Data-parallel over 8 NeuronCores: batch 1024 -> 128 per core.
"""

import os
import numpy as np

import concourse.bacc as bacc
import concourse.mybir as mybir
import concourse.tile as tile
from concourse.bass_utils import run_bass_kernel_spmd

N_CORES = 8
B, D, VIEW, C = 1024, 256, 4, 16
BL = B // N_CORES  # 128 samples per core
F32 = mybir.dt.float32
F16 = mybir.dt.float16

AOP = mybir.AluOpType
AFT = mybir.ActivationFunctionType
AXL = mybir.AxisListType

_cache = {}


def _build(xbufs: int = 6, yybufs: int = 2):
    key = (xbufs, yybufs)
    if key in _cache:
        return _cache[key]

    nc = bacc.Bacc("TRN2", target_bir_lowering=False, debug=False,
                   num_devices=N_CORES)

    x_d = nc.dram_tensor("x", [128, BL, 2, 256], F16, kind="ExternalInput").ap()
    ca_d = nc.dram_tensor("ca", [128, 640], F16, kind="ExternalInput").ap()
    cc_d = nc.dram_tensor("cc", [128, 1536], F16, kind="ExternalInput").ap()
    cb_d = nc.dram_tensor("cb", [128, 176], F32, kind="ExternalInput").ap()
    out_d = nc.dram_tensor("probs", [BL, C], F32, kind="ExternalOutput").ap()

    with tile.TileContext(nc) as tc:
        with (
            tc.tile_pool(name="consts", bufs=1) as consts,
            tc.tile_pool(name="xp", bufs=xbufs) as xpool,
            tc.tile_pool(name="xr", bufs=2) as xrpool,
            tc.tile_pool(name="z1", bufs=2) as z1pool,
            tc.tile_pool(name="scr", bufs=3) as scrpool,
            tc.tile_pool(name="fin", bufs=1) as fin,
            tc.tile_pool(name="yy", bufs=yybufs, space="PSUM") as yypool,
            tc.tile_pool(name="y1", bufs=1, space="PSUM") as y1pool,
            tc.tile_pool(name="sc", bufs=1, space="PSUM") as scpool,
        ):
            # macro-blocks of 8 samples (stage-1b / path-1 batch size);
            # y14 [128, 8, 64] f32 is exactly one PSUM bank (matmul output
            # must not cross banks)
            _m = int(os.environ.get("NN_MACRO", "8"))
            macros = [_m] * (BL // _m)
            mstarts = [sum(macros[:i]) for i in range(len(macros))]

            # first two X loads before anything else so DMA starts at t=0
            pre = {}
            for grp in range(2):
                xt = xpool.tile([128, 4, 2, 256], F16, tag="xt")
                nc.sync.dma_start(out=xt, in_=x_d[:, grp * 4:grp * 4 + 4])
                pre[grp] = xt

            # ---- constants (3 packed DMAs) ----
            # ca: [vp (2 parity x 2 mc x 128) | v1s (128)]
            # cc: [v2q (2x256, rows 0:64) | v1q16 (16x64)]
            # cb: [g1 (16) | g2 dup rows 0:32/32:64 (16) | bo (144)]
            ca = consts.tile([128, 640], F16)
            nc.sync.dma_start(out=ca, in_=ca_d)
            cc = consts.tile([128, 1536], F16)
            nc.sync.dma_start(out=cc, in_=cc_d)
            cb = consts.tile([128, 176], F32)
            nc.sync.dma_start(out=cb, in_=cb_d)
            vp = ca[:, 0:512].rearrange("p (par m c) -> p par m c", par=2, m=2)
            v1s = ca[:, 512:640]
            v2q = cc[0:64, 0:512].rearrange("p (s n) -> p s n", s=2)
            v1q = cc[:, 512:1536].rearrange("p (s j) -> p s j", s=16)
            g1 = cb[:, 0:16]
            g2a = cb[0:32, 16:32]
            g2b = cb[32:64, 16:32]
            bo = cb[0:1, 32:176]

            r1 = consts.tile([128, BL], F32)    # path-1 per-k partial scores
            r2z = consts.tile([64, 129], F32)   # path-2, data in odd cols
            if os.environ.get("NN_MS", "pool") == "pool":
                nc.gpsimd.memset(r2z, 0.0)
            else:
                nc.vector.memset(r2z, 0.0)

            grp = 0
            for mi, ms in enumerate(macros):
                s0 = mstarts[mi]
                xr16 = xrpool.tile([128, ms, 256], F16, tag="xr16")
                for pl in range(ms // 2):
                    if pl % 2 == 0:
                        if grp in pre:
                            xt = pre[grp]
                        else:
                            xt = xpool.tile([128, 4, 2, 256], F16, tag="xt")
                            nc.sync.dma_start(
                                out=xt, in_=x_d[:, grp * 4:grp * 4 + 4])
                        grp += 1
                    p_global = (s0 + 2 * pl) // 2
                    yy2 = yypool.tile([128, 2, 256], F32)
                    for si2 in range(2):
                        si = (2 * pl + si2) % 4  # index within the DMA group
                        nc.tensor.matmul(yy2[:, si2, :], vp[:, si2, 0, :],
                                         xt[:, si, 0, :],
                                         start=True, stop=False)
                        nc.tensor.matmul(yy2[:, si2, :], vp[:, si2, 1, :],
                                         xt[:, si, 1, :],
                                         start=False, stop=True)
                    # one pair-batched PSUM->SBUF fp16 copy (ACT cost is
                    # free-size only, so the Y2 rows ride along with Xr)
                    nc.scalar.copy(xr16[:, 2 * pl:2 * pl + 2, :], yy2)
                    # path 2: both samples of the pair in ONE fused op
                    if os.environ.get("NN_P2", "pair") == "pair":
                        scr2 = scrpool.tile([64, 2, 256], F16, tag="scr2")
                        nc.vector.scalar_tensor_tensor(
                            out=scr2, in0=xr16[0:64, 2 * pl:2 * pl + 2, :],
                            scalar=1.0, in1=v2q, op0=AOP.mult, op1=AOP.mult,
                            accum_out=r2z[:, 2 * p_global + 1:
                                          2 * p_global + 2])

                # stage 1b: Y1 = V1^T Xrp, col-pool via 4 accumulated matmuls,
                # all ms samples batched in the moving operand
                y14 = y1pool.tile([128, ms, 64], F32, tag="y14")
                for q in range(4):
                    nc.tensor.matmul(y14, v1s[64:128, :],
                                     xr16[64:128, :, q * 64:(q + 1) * 64],
                                     start=(q == 0), stop=(q == 3))
                # path 1: batched multiply then per-sample reduce over j
                if os.environ.get("NN_P1", "tt") == "tt":
                    z1 = z1pool.tile([128, ms, 64], F16, tag="z1")
                    nc.vector.tensor_tensor(out=z1, in0=y14,
                                            in1=v1q[:, 0:ms, :], op=AOP.mult)
                    nc.vector.tensor_reduce(out=r1[:, s0:s0 + ms], in_=z1,
                                            axis=AXL.X, op=AOP.add)
                else:
                    for si in range(ms):
                        scr1 = scrpool.tile([128, 64], F32, tag="scr1")
                        nc.vector.scalar_tensor_tensor(
                            out=scr1, in0=y14[:, si, :], scalar=1.0,
                            in1=v1q[:, 0, :], op0=AOP.mult, op1=AOP.mult,
                            accum_out=r1[:, s0 + si:s0 + si + 1])

            # ---- stage 3: scores [128 samples, 16] ----
            sc = scpool.tile([BL, C], F32)
            nc.tensor.matmul(sc, r1, g1, start=True, stop=False)
            if os.environ.get("NN_S3", "r2z") == "r2z":
                nc.tensor.matmul(sc, r2z[0:32, 1:129], g2a,
                                 start=False, stop=False)
                nc.tensor.matmul(sc, r2z[32:64, 0:128], g2b,
                                 start=False, stop=False)
            nc.tensor.matmul(sc, bo[:, 0:128], bo[:, 128:144],
                             start=False, stop=True)

            # ---- softmax over the 16 free elements ----
            negmax = fin.tile([BL, 1], F32)
            nc.vector.tensor_reduce(out=negmax, in_=sc, axis=AXL.X,
                                    op=AOP.max, negate=True)
            e = fin.tile([BL, C], F32)
            sume = fin.tile([BL, 1], F32)
            nc.scalar.activation(out=e, in_=sc, func=AFT.Exp, bias=negmax,
                                 scale=1.0, accum_out=sume)
            rec = fin.tile([BL, 1], F32)
            nc.vector.reciprocal(rec, sume)
            probs = fin.tile([BL, C], F32)
            nc.vector.tensor_scalar_mul(probs, e, rec)
            nc.sync.dma_start(out=out_d, in_=probs)

    nc.compile()
    _cache[key] = nc
    return nc


def _host_prep(inputs, w1, w2, l, b):
    inputs = np.asarray(inputs, dtype=np.float32)
    w1 = np.asarray(w1, dtype=np.float32)
    w2 = np.asarray(w2, dtype=np.float32)
    l = np.asarray(l, dtype=np.float32)
    b = np.asarray(b, dtype=np.float32)

    g = float(1.0 / (1.0 + np.exp(-np.float32(l[0]))))

    # path 1: rank-8 factors on the 64-block; col k = cls*8 + r
    u1, v1 = w1[:, :, 4:], w1[:, :, :4]                # [16, 64, 4]
    v1cols = np.concatenate([u1, v1], axis=2)          # [16, 64, 8]
    v1small = np.ascontiguousarray(
        v1cols.transpose(1, 0, 2).reshape(64, 128)).astype(np.float32)
    s1 = np.tile(np.array([g] * 4 + [-g] * 4, np.float32), C)        # [128]
    v1pt = np.ascontiguousarray(v1small.T * s1[:, None]).astype(np.float32)

    # path 2: rank-2 factors on full d; col k = cls*2 + {u, v}
    u2, v2 = w2[:, :, 1:2], w2[:, :, 0:1]              # [16, 256, 1]
    v2cols = np.concatenate([u2, v2], axis=2)          # [16, 256, 2]
    v2full = np.ascontiguousarray(
        v2cols.transpose(1, 0, 2).reshape(256, 32)).astype(np.float32)
    s2 = np.tile(np.array([1.0 - g, -(1.0 - g)], np.float32), C)     # [32]
    v2pt = np.ascontiguousarray(v2full.T * s2[:, None]).astype(np.float32)

    # stage-1 stationaries, one per sample parity:
    #   parity 0: [V2 | 0 | P],  parity 1: [0 | V2 | P]
    P = (np.arange(128)[:, None] % 64 == np.arange(64)[None, :]).astype(
        np.float32)
    vp = np.zeros((128, 2, 2, 128), np.float32)        # [p, parity, mc, c]
    for par in range(2):
        for mc in range(2):
            vp[:, par, mc, 32 * par:32 * par + 32] = \
                v2full[mc * 128:(mc + 1) * 128, :]
            vp[:, par, mc, 64:128] = P

    # path-2 DVE weights, stacked for the pair: rows 0:32 even, 32:64 odd
    v2weights = np.zeros((64, 2, 256), np.float32)
    v2weights[0:32, 0, :] = v2pt
    v2weights[32:64, 1, :] = v2pt

    g1m = (np.arange(128)[:, None] // 8 ==
           np.arange(C)[None, :]).astype(np.float32)
    g2m = (np.arange(32)[:, None] // 2 ==
           np.arange(C)[None, :]).astype(np.float32)

    ca = np.zeros((128, 640), np.float16)
    ca[:, 0:512] = vp.reshape(128, 512).astype(np.float16)
    ca[64:128, 512:640] = v1small.astype(np.float16)

    cc = np.zeros((128, 1536), np.float16)
    cc[0:64, 0:512] = v2weights.reshape(64, 512).astype(np.float16)
    cc[:, 512:1536] = np.tile(v1pt[:, None, :], (1, 16, 1)).reshape(
        128, 1024).astype(np.float16)

    cb = np.zeros((128, 176), np.float32)
    cb[:, 0:16] = g1m
    cb[0:32, 16:32] = g2m
    cb[32:64, 16:32] = g2m
    cb[0, 32:160] = 1.0
    cb[0, 160:176] = b

    # shard + relayout inputs: (core, p, s, mc, n), cast to fp16
    xs = inputs.reshape(N_CORES, BL, 2, 128, 256).transpose(0, 3, 1, 2, 4)

    shared = dict(ca=ca, cb=cb, cc=cc)
    in_maps = []
    for core in range(N_CORES):
        m = dict(shared)
        m["x"] = np.ascontiguousarray(xs[core]).astype(np.float16)
        in_maps.append(m)
    return in_maps


def kernel(inputs, w1, w2, l, b, _trace=False, _mm_fast=None):
    nc = _build()
    in_maps = _host_prep(inputs, w1, w2, l, b)
    res = run_bass_kernel_spmd(nc, in_maps, core_ids=list(range(N_CORES)),
                               trace=_trace)
    out = np.concatenate([r["probs"] for r in res.results], axis=0)
    if _trace:
        kernel.last_results = res
    return out
